# revision 4
# baseline (speedup 1.0000x reference)
"""Trainium2 kernel for the nn_Circuit coupled-mode ODE problem.

Math: dA/dt = i*diag(omega + gamma*|A|^2) A + T2 A, integrated from t=0 to 2,
sampled at 200 points, with A (1024 batch, 64 modes) complex, A0 padding to
ones for modes 48..63.  T2 (and hence L = T2 + i*diag(omega)) is constant and
nearly skew-Hermitian, with one stiff oscillatory eigenvalue (~288i).

Device algorithm: Strang splitting with the linear part applied EXACTLY via a
host-precomputed matrix exponential (one 128x128 real matmul per step) and the
nonlinear part applied exactly as a per-element phase rotation
A <- A*exp(i*gamma*h*|A|^2) (sin/cos on the scalar engine).  One step per
output interval (h = 2/199).  Using the half-shifted chain state
z_k = E(h/2) y_k:
    u_k    = NL_h(z_k)
    y_{k+1} = E(h/2) u_k          (output branch, transposed+interleaved matmul)
    z_{k+1} = E(h)   u_k          (chain matmul)

State layout on device: (128 partitions, 128 batch) f32 with partition
p = 2j+c interleaving re/im of mode j, so that |A|^2 needs only a pair-swap
stream_shuffle, and the output matmul directly produces the DRAM (batch, j, c)
layout.

Sharding: pure data parallel, batch 1024 = 8 cores x 128.
"""

import os
import numpy as np

MODES = 64
INPUT_MODES = 48
BATCH = 1024
EVAL_PTS = 200
EPS = 1e-8
N_CORES = 8
B_LOC = BATCH // N_CORES  # 128
NT = EVAL_PTS - 1  # 199 intervals
DT = 2.0 / NT

_CACHE = {}


# ---------------------------------------------------------------------------
# host-side math
# ---------------------------------------------------------------------------

def _t2_like_reference(params, omega, kappa):
    """Reproduce the reference's float32 jax computation of T2 exactly."""
    import jax

    try:
        cpu = jax.devices("cpu")[0]
    except Exception:
        cpu = None

    import contextlib

    ctx = jax.default_device(cpu) if cpu is not None else contextlib.nullcontext()
    with ctx:
        import jax.numpy as jnp

        n = MODES
        p = jnp.asarray(params, dtype=jnp.float32)
        n_off = n * (n - 1) // 2
        iu = jnp.triu_indices(n, 1)
        off = p[:n_off] + 1j * p[n_off:2 * n_off]
        H = jnp.zeros((n, n), dtype=jnp.complex64).at[iu].set(off.astype(jnp.complex64))
        H = H + H.conj().T
        d = p[2 * n_off:]
        diag = jnp.concatenate([d, -jnp.sum(d, keepdims=True)])
        H = H + jnp.diag(diag.astype(jnp.complex64))
        U = jax.scipy.linalg.expm(1j * H)
        I = jnp.eye(n, dtype=jnp.complex64)
        M = U.T @ U
        mix = M @ jnp.linalg.inv(I - M + EPS * I)
        T2 = -jnp.asarray(kappa, dtype=jnp.float32) * (
            0.5 * jnp.eye(n, dtype=jnp.float32) + mix
        )
        T2_re = np.asarray(jnp.real(T2), dtype=np.float32)
        T2_im = np.asarray(jnp.imag(T2), dtype=np.float32)
    return T2_re, T2_im


def _expm(M):
    """Matrix exponential of a (diagonalizable) complex matrix via eig."""
    w, V = np.linalg.eig(M)
    return (V * np.exp(w)) @ np.linalg.inv(V)


def _big_il(C):
    """Complex (64,64) -> real (128,128) operator in the interleaved re/im basis."""
    A = np.zeros((2 * MODES, 2 * MODES), dtype=np.float64)
    Cr, Ci = C.real, C.imag
    A[0::2, 0::2] = Cr
    A[0::2, 1::2] = -Ci
    A[1::2, 0::2] = Ci
    A[1::2, 1::2] = Cr
    return A


def _host_precompute(A0, params, omega, kappa, nonlinearity):
    T2_re, T2_im = _t2_like_reference(params, omega, kappa)
    L = T2_re.astype(np.float64) + 1j * T2_im.astype(np.float64)
    L = L + 1j * np.diag(omega.astype(np.float64))

    E1 = _expm(L * DT)          # full-step propagator
    E2 = _expm(L * (DT / 2))    # half-step propagator

    A1 = _big_il(E1)
    A2 = _big_il(E2)
    perm = np.arange(128) ^ 1   # pair swap

    wE = np.ascontiguousarray(A1.T, dtype=np.float32)
    wEsw = np.ascontiguousarray(A1[perm, :].T, dtype=np.float32)
    mOut = np.ascontiguousarray(A2.T, dtype=np.float32)

    # initial state, interleaved: (128, BATCH)
    y0 = np.zeros((2 * MODES, BATCH), dtype=np.float64)
    y0[0:2 * INPUT_MODES:2, :] = A0[:, :, 0].astype(np.float64).T
    y0[1:2 * INPUT_MODES:2, :] = A0[:, :, 1].astype(np.float64).T
    y0[2 * INPUT_MODES::2, :] = 1.0
    z0 = (A2 @ y0).astype(np.float32)
    y0T = np.ascontiguousarray(y0.T, dtype=np.float32)  # (BATCH, 128)

    gh = (nonlinearity.astype(np.float64) * DT)  # per-mode gamma*h
    cosscale = np.repeat(gh, 2).astype(np.float32).reshape(128, 1)
    sgn = np.tile([-1.0, 1.0], MODES)
    sinscale = (np.repeat(gh, 2) * sgn).astype(np.float32).reshape(128, 1)

    return dict(wE=wE, wEsw=wEsw, mOut=mOut, z0=z0, y0T=y0T,
                cosscale=cosscale, sinscale=sinscale)


# ---------------------------------------------------------------------------
# device kernel
# ---------------------------------------------------------------------------

def _build_nc():
    import concourse.bass as bass
    import concourse.bacc as bacc
    import concourse.tile as tile
    import concourse.mybir as mybir

    f32 = mybir.dt.float32
    Sin = mybir.ActivationFunctionType.Sin
    Square = mybir.ActivationFunctionType.Square
    Copy = mybir.ActivationFunctionType.Copy
    add = mybir.AluOpType.add
    mult = mybir.AluOpType.mult
    P = 128
    pairswap = [i ^ 1 for i in range(32)]

    nc = bacc.Bacc("TRN2", target_bir_lowering=False, debug=False,
                   num_devices=N_CORES)

    wE_d = nc.dram_tensor("wE", [P, P], f32, kind="ExternalInput").ap()
    wEsw_d = nc.dram_tensor("wEsw", [P, P], f32, kind="ExternalInput").ap()
    mOut_d = nc.dram_tensor("mOut", [P, P], f32, kind="ExternalInput").ap()
    z0_d = nc.dram_tensor("z0", [P, B_LOC], f32, kind="ExternalInput").ap()
    y0T_d = nc.dram_tensor("y0T", [B_LOC, P], f32, kind="ExternalInput").ap()
    cosscale_d = nc.dram_tensor("cosscale", [P, 1], f32, kind="ExternalInput").ap()
    sinscale_d = nc.dram_tensor("sinscale", [P, 1], f32, kind="ExternalInput").ap()
    out_d = nc.dram_tensor("out", [EVAL_PTS, B_LOC, P], f32, kind="ExternalOutput").ap()

    GROUPS = [(0, 64), (64, 128)]

    with tile.TileContext(nc) as tc:
        with (
            tc.tile_pool(name="const", bufs=1) as cpool,
            tc.tile_pool(name="u", bufs=3) as upool,
            tc.tile_pool(name="nl", bufs=3) as npool,
            tc.tile_pool(name="oy", bufs=4) as opool,
            tc.tile_pool(name="pz", bufs=2, space="PSUM") as pzpool,
            tc.tile_pool(name="pw", bufs=2, space="PSUM") as pwpool,
            tc.tile_pool(name="py", bufs=2, space="PSUM") as pypool,
        ):
            wE_t = cpool.tile([P, P], f32, tag="wE")
            wEsw_t = cpool.tile([P, P], f32, tag="wEsw")
            mOut_t = cpool.tile([P, P], f32, tag="mOut")
            cossc_t = cpool.tile([P, 1], f32, tag="cossc")
            sinsc_t = cpool.tile([P, 1], f32, tag="sinsc")
            bias_t = cpool.tile([P, 1], f32, tag="bias")
            nc.sync.dma_start(wE_t[:], wE_d[:])
            nc.sync.dma_start(wEsw_t[:], wEsw_d[:])
            nc.sync.dma_start(mOut_t[:], mOut_d[:])
            nc.sync.dma_start(cossc_t[:], cosscale_d[:])
            nc.sync.dma_start(sinsc_t[:], sinscale_d[:])
            nc.vector.memset(bias_t[:], float(np.pi / 2))

            # t=0 output: pass-through of the initial state
            y0_t = opool.tile([B_LOC, P], f32, tag="yc")
            nc.sync.dma_start(y0_t[:], y0T_d[:])
            nc.sync.dma_start(out_d[0], y0_t[:])

            # ---- initial nonlinear step: u_0 = NL(z0), z0 from DRAM ----
            z0_t = upool.tile([P, B_LOC], f32, tag="z0src")
            nc.sync.dma_start(z0_t[:], z0_d[:])
            u = upool.tile([P, B_LOC], f32, tag="u")
            s2i = npool.tile([P, B_LOC], f32, tag="s2i")
            s2swi = npool.tile([P, B_LOC], f32, tag="s2swi")
            m2i = npool.tile([P, B_LOC], f32, tag="m2i")
            CCi = npool.tile([P, B_LOC], f32, tag="CCi")
            SSi = npool.tile([P, B_LOC], f32, tag="SSi")
            zswi = npool.tile([P, B_LOC], f32, tag="zswi")
            pi_ = npool.tile([P, B_LOC], f32, tag="pi_")
            qi_ = npool.tile([P, B_LOC], f32, tag="qi_")
            nc.vector.tensor_tensor(s2i[:], z0_t[:], z0_t[:], mult)
            nc.vector.stream_shuffle(s2swi[:], s2i[:], pairswap)
            nc.gpsimd.tensor_tensor(m2i[:], s2i[:], s2swi[:], add)
            nc.scalar.activation(CCi[:], m2i[:], Sin, bias=bias_t[:], scale=cossc_t[:])
            nc.scalar.activation(SSi[:], m2i[:], Sin, scale=sinsc_t[:])
            nc.vector.stream_shuffle(zswi[:], z0_t[:], pairswap)
            nc.gpsimd.tensor_tensor(pi_[:], z0_t[:], CCi[:], mult)
            nc.vector.tensor_tensor(qi_[:], zswi[:], SSi[:], mult)
            nc.gpsimd.tensor_tensor(u[:], pi_[:], qi_[:], add)

            # ---- main loop ----
            # PSUM single-reading-engine rule (ACT+DVE on the same PSUM bank in
            # parallel is a HW fault): z is read only by the V copy zc, zsw only
            # by the V muls qq, yps only by the ACT copy yc.
            for k in range(NT):
                # output branch: yT = (u as stationary) x mOut -> (batch, 2j+c)
                yps = pypool.tile([B_LOC, P], f32, tag="yps")
                nc.tensor.matmul(yps[:], u[:], mOut_t[:], start=True, stop=True)
                yc = opool.tile([B_LOC, P], f32, tag="yc")
                nc.scalar.activation(yc[:], yps[:], Copy)
                nc.sync.dma_start(out_d[k + 1], yc[:])

                if k == NT - 1:
                    break

                # chain matmuls (full width)
                z = pzpool.tile([P, B_LOC], f32, tag="z")
                zsw = pwpool.tile([P, B_LOC], f32, tag="zsw")
                nc.tensor.matmul(z[:], wE_t[:], u[:], start=True, stop=True)
                nc.tensor.matmul(zsw[:], wEsw_t[:], u[:], start=True, stop=True)

                zc = npool.tile([P, B_LOC], f32, tag="zc")
                nc.vector.tensor_copy(zc[:], z[:])

                unew = upool.tile([P, B_LOC], f32, tag="u")
                for g, (c0, c1) in enumerate(GROUPS):
                    s2 = npool.tile([P, 64], f32, tag=f"s2_{g}")
                    s2sw = npool.tile([P, 64], f32, tag=f"s2sw_{g}")
                    m2 = npool.tile([P, 64], f32, tag=f"m2_{g}")
                    CC = npool.tile([P, 64], f32, tag=f"CC_{g}")
                    SS = npool.tile([P, 64], f32, tag=f"SS_{g}")
                    pp = npool.tile([P, 64], f32, tag=f"pp_{g}")
                    qq = npool.tile([P, 64], f32, tag=f"qq_{g}")
                    nc.vector.tensor_tensor(s2[:], zc[:, c0:c1], zc[:, c0:c1], mult)
                    nc.vector.stream_shuffle(s2sw[:], s2[:], pairswap)
                    nc.gpsimd.tensor_tensor(m2[:], s2[:], s2sw[:], add)
                    nc.scalar.activation(CC[:], m2[:], Sin, bias=bias_t[:], scale=cossc_t[:])
                    nc.scalar.activation(SS[:], m2[:], Sin, scale=sinsc_t[:])
                    nc.gpsimd.tensor_tensor(pp[:], zc[:, c0:c1], CC[:], mult)
                    nc.vector.tensor_tensor(qq[:], zsw[:, c0:c1], SS[:], mult)
                    nc.gpsimd.tensor_tensor(unew[:, c0:c1], pp[:], qq[:], add)
                u = unew

    nc.compile()
    return nc


def _get_compiled():
    if "nc" not in _CACHE:
        _CACHE["nc"] = _build_nc()
    return _CACHE["nc"]


def _run(host, trace=False, tmpdir=None):
    from concourse.bass_utils import run_bass_kernel_spmd

    nc = _get_compiled()
    in_maps = []
    for i in range(N_CORES):
        sl = slice(i * B_LOC, (i + 1) * B_LOC)
        in_maps.append({
            "wE": host["wE"],
            "wEsw": host["wEsw"],
            "mOut": host["mOut"],
            "z0": np.ascontiguousarray(host["z0"][:, sl]),
            "y0T": np.ascontiguousarray(host["y0T"][sl, :]),
            "cosscale": host["cosscale"],
            "sinscale": host["sinscale"],
        })
    res = run_bass_kernel_spmd(nc, in_maps, list(range(N_CORES)), trace=trace,
                               tmpdir=tmpdir)
    full = np.empty((EVAL_PTS, BATCH, MODES, 2), dtype=np.float32)
    for i in range(N_CORES):
        sl = slice(i * B_LOC, (i + 1) * B_LOC)
        full[:, sl, :, :] = res.results[i]["out"].reshape(EVAL_PTS, B_LOC, MODES, 2)
    return full, res


def kernel(A0, params, omega, kappa, nonlinearity):
    A0 = np.asarray(A0, dtype=np.float32)
    params = np.asarray(params, dtype=np.float32)
    omega = np.asarray(omega, dtype=np.float32)
    kappa = np.asarray(kappa, dtype=np.float32)
    nonlinearity = np.asarray(nonlinearity, dtype=np.float32)

    host = _host_precompute(A0, params, omega, kappa, nonlinearity)
    full, _ = _run(host, trace=False)
    return full


# revision 7
# speedup vs baseline: 1.1771x; 1.1771x over previous
"""Trainium2 kernel for the nn_Circuit coupled-mode ODE problem.

Math: dA/dt = i*diag(omega + gamma*|A|^2) A + T2 A, integrated from t=0 to 2,
sampled at 200 points, with A (1024 batch, 64 modes) complex, A0 padding to
ones for modes 48..63.  T2 (and hence L = T2 + i*diag(omega)) is constant and
nearly skew-Hermitian, with one stiff oscillatory eigenvalue (~288i).

Device algorithm: Strang splitting with the linear part applied EXACTLY via a
host-precomputed matrix exponential (one 128x128 real matmul per step) and the
nonlinear part applied exactly as a per-element phase rotation
A <- A*exp(i*gamma*h*|A|^2) (sin/cos on the scalar engine).  One step per
output interval (h = 2/199).  Using the half-shifted chain state
z_k = E(h/2) y_k:
    u_k    = NL_h(z_k)
    y_{k+1} = E(h/2) u_k          (output branch, transposed+interleaved matmul)
    z_{k+1} = E(h)   u_k          (chain matmul)

State layout on device: (128 partitions, 128 batch) f32 with partition
p = 2j+c interleaving re/im of mode j, so that |A|^2 needs only a pair-swap
stream_shuffle, and the output matmul directly produces the DRAM (batch, j, c)
layout.

Sharding: pure data parallel, batch 1024 = 8 cores x 128.
"""

import os
import numpy as np

MODES = 64
INPUT_MODES = 48
BATCH = 1024
EVAL_PTS = 200
EPS = 1e-8
N_CORES = 8
B_LOC = BATCH // N_CORES  # 128
NT = EVAL_PTS - 1  # 199 intervals
DT = 2.0 / NT

_CACHE = {}


# ---------------------------------------------------------------------------
# host-side math
# ---------------------------------------------------------------------------

def _t2_like_reference(params, omega, kappa):
    """Reproduce the reference's float32 jax computation of T2 exactly."""
    import jax

    try:
        cpu = jax.devices("cpu")[0]
    except Exception:
        cpu = None

    import contextlib

    ctx = jax.default_device(cpu) if cpu is not None else contextlib.nullcontext()
    with ctx:
        import jax.numpy as jnp

        n = MODES
        p = jnp.asarray(params, dtype=jnp.float32)
        n_off = n * (n - 1) // 2
        iu = jnp.triu_indices(n, 1)
        off = p[:n_off] + 1j * p[n_off:2 * n_off]
        H = jnp.zeros((n, n), dtype=jnp.complex64).at[iu].set(off.astype(jnp.complex64))
        H = H + H.conj().T
        d = p[2 * n_off:]
        diag = jnp.concatenate([d, -jnp.sum(d, keepdims=True)])
        H = H + jnp.diag(diag.astype(jnp.complex64))
        U = jax.scipy.linalg.expm(1j * H)
        I = jnp.eye(n, dtype=jnp.complex64)
        M = U.T @ U
        mix = M @ jnp.linalg.inv(I - M + EPS * I)
        T2 = -jnp.asarray(kappa, dtype=jnp.float32) * (
            0.5 * jnp.eye(n, dtype=jnp.float32) + mix
        )
        T2_re = np.asarray(jnp.real(T2), dtype=np.float32)
        T2_im = np.asarray(jnp.imag(T2), dtype=np.float32)
    return T2_re, T2_im


def _expm(M):
    """Matrix exponential of a (diagonalizable) complex matrix via eig."""
    w, V = np.linalg.eig(M)
    return (V * np.exp(w)) @ np.linalg.inv(V)


def _big_il(C):
    """Complex (64,64) -> real (128,128) operator in the interleaved re/im basis."""
    A = np.zeros((2 * MODES, 2 * MODES), dtype=np.float64)
    Cr, Ci = C.real, C.imag
    A[0::2, 0::2] = Cr
    A[0::2, 1::2] = -Ci
    A[1::2, 0::2] = Ci
    A[1::2, 1::2] = Cr
    return A


def _host_precompute(A0, params, omega, kappa, nonlinearity):
    T2_re, T2_im = _t2_like_reference(params, omega, kappa)
    L = T2_re.astype(np.float64) + 1j * T2_im.astype(np.float64)
    L = L + 1j * np.diag(omega.astype(np.float64))

    E1 = _expm(L * DT)          # full-step propagator
    E2 = _expm(L * (DT / 2))    # half-step propagator

    A1 = _big_il(E1)
    A2 = _big_il(E2)
    perm = np.arange(128) ^ 1   # pair swap

    wE = np.ascontiguousarray(A1.T, dtype=np.float32)
    wEsw = np.ascontiguousarray(A1[perm, :].T, dtype=np.float32)
    mOut = np.ascontiguousarray(A2.T, dtype=np.float32)

    # initial state, interleaved: (128, BATCH)
    y0 = np.zeros((2 * MODES, BATCH), dtype=np.float64)
    y0[0:2 * INPUT_MODES:2, :] = A0[:, :, 0].astype(np.float64).T
    y0[1:2 * INPUT_MODES:2, :] = A0[:, :, 1].astype(np.float64).T
    y0[2 * INPUT_MODES::2, :] = 1.0
    z0 = (A2 @ y0).astype(np.float32)
    y0T = np.ascontiguousarray(y0.T, dtype=np.float32)  # (BATCH, 128)

    gh = (nonlinearity.astype(np.float64) * DT)  # per-mode gamma*h
    # SSp = sin(theta) with sign (+ on even partitions, - on odd): the rotation
    # cross-term is built as q = pairswap(z * SSp).
    sgn = np.tile([1.0, -1.0], MODES)
    sinscale = (np.repeat(gh, 2) * sgn).astype(np.float32).reshape(128, 1)
    # cc = 1 - theta^2/2 = msq * (-(gamma*h)^2/2) + 1, per-partition coefficient
    ccscale = (-np.repeat(gh, 2) ** 2 / 2).astype(np.float32).reshape(128, 1)

    return dict(wE=wE, wEsw=wEsw, mOut=mOut, z0=z0, y0T=y0T,
                ccscale=ccscale, sinscale=sinscale)


# ---------------------------------------------------------------------------
# device kernel
# ---------------------------------------------------------------------------

def _build_nc():
    import concourse.bass as bass
    import concourse.bacc as bacc
    import concourse.tile as tile
    import concourse.mybir as mybir

    f32 = mybir.dt.float32
    Sin = mybir.ActivationFunctionType.Sin
    Square = mybir.ActivationFunctionType.Square
    Copy = mybir.ActivationFunctionType.Copy
    add = mybir.AluOpType.add
    mult = mybir.AluOpType.mult
    P = 128
    pairswap = [i ^ 1 for i in range(32)]

    nc = bacc.Bacc("TRN2", target_bir_lowering=False, debug=False,
                   num_devices=N_CORES)

    wE_d = nc.dram_tensor("wE", [P, P], f32, kind="ExternalInput").ap()
    mOut_d = nc.dram_tensor("mOut", [P, P], f32, kind="ExternalInput").ap()
    z0_d = nc.dram_tensor("z0", [P, B_LOC], f32, kind="ExternalInput").ap()
    y0T_d = nc.dram_tensor("y0T", [B_LOC, P], f32, kind="ExternalInput").ap()
    ccscale_d = nc.dram_tensor("ccscale", [P, 1], f32, kind="ExternalInput").ap()
    sinscale_d = nc.dram_tensor("sinscale", [P, 1], f32, kind="ExternalInput").ap()
    out_d = nc.dram_tensor("out", [EVAL_PTS, B_LOC, P], f32, kind="ExternalOutput").ap()

    GROUPS = [(0, 64), (64, 128)]

    with tile.TileContext(nc) as tc:
        with (
            tc.tile_pool(name="const", bufs=1) as cpool,
            tc.tile_pool(name="u", bufs=3) as upool,
            tc.tile_pool(name="nl", bufs=3) as npool,
            tc.tile_pool(name="oy", bufs=4) as opool,
            tc.tile_pool(name="pz0", bufs=2, space="PSUM") as pz0pool,
            tc.tile_pool(name="pz1", bufs=2, space="PSUM") as pz1pool,
            tc.tile_pool(name="py", bufs=2, space="PSUM") as pypool,
        ):
            wE_t = cpool.tile([P, P], f32, tag="wE")
            mOut_t = cpool.tile([P, P], f32, tag="mOut")
            ccsc_t = cpool.tile([P, 1], f32, tag="ccsc")
            sinsc_t = cpool.tile([P, 1], f32, tag="sinsc")
            nc.sync.dma_start(wE_t[:], wE_d[:])
            nc.sync.dma_start(mOut_t[:], mOut_d[:])
            nc.sync.dma_start(ccsc_t[:], ccscale_d[:])
            nc.sync.dma_start(sinsc_t[:], sinscale_d[:])

            # t=0 output: pass-through of the initial state
            y0_t = opool.tile([B_LOC, P], f32, tag="yc")
            nc.sync.dma_start(y0_t[:], y0T_d[:])
            nc.sync.dma_start(out_d[0], y0_t[:])

            pzpools = [pz0pool, pz1pool]

            def nl_group(g, c0, c1, zsrc, unew, from_sbuf):
                """u'[:, c0:c1] = exp(i*theta)*z from zsrc (slice or tile)."""
                FD = c1 - c0
                s2 = npool.tile([P, FD], f32, tag=f"s2_{g}")
                s2sw = npool.tile([P, FD], f32, tag=f"s2sw_{g}")
                m2 = npool.tile([P, FD], f32, tag=f"m2_{g}")
                msq = npool.tile([P, FD], f32, tag=f"msq_{g}")
                cc = npool.tile([P, FD], f32, tag=f"cc_{g}")
                ssp = npool.tile([P, FD], f32, tag=f"ssp_{g}")
                pp = npool.tile([P, FD], f32, tag=f"pp_{g}")
                qt = npool.tile([P, FD], f32, tag=f"qt_{g}")
                qq = npool.tile([P, FD], f32, tag=f"qq_{g}")
                if from_sbuf:
                    nc.vector.tensor_tensor(s2[:], zsrc, zsrc, mult)
                else:
                    nc.scalar.activation(s2[:], zsrc, Square)
                nc.vector.stream_shuffle(s2sw[:], s2[:], pairswap)
                nc.gpsimd.tensor_tensor(m2[:], s2[:], s2sw[:], add)
                nc.scalar.activation(ssp[:], m2[:], Sin, scale=sinsc_t[:])
                nc.vector.tensor_tensor(msq[:], m2[:], m2[:], mult)
                nc.vector.tensor_scalar(cc[:], msq[:], ccsc_t[:], 1.0, mult, add)
                nc.vector.tensor_tensor(pp[:], zsrc, cc[:], mult)
                nc.vector.tensor_tensor(qt[:], zsrc, ssp[:], mult)
                nc.vector.stream_shuffle(qq[:], qt[:], pairswap)
                nc.gpsimd.tensor_tensor(unew[:, c0:c1], pp[:], qq[:], add)

            # ---- initial nonlinear step: u_0 = NL(z0), z0 from DRAM ----
            z0_t = upool.tile([P, B_LOC], f32, tag="z0src")
            nc.sync.dma_start(z0_t[:], z0_d[:])
            u = upool.tile([P, B_LOC], f32, tag="u")
            for g, (c0, c1) in enumerate(GROUPS):
                nl_group(g, c0, c1, z0_t[:, c0:c1], u, from_sbuf=True)

            # ---- main loop: two independent per-group chains ----
            # PSUM single-reader discipline: each z_g PSUM tile is read by ACT
            # (Square) first and V (pp/qt muls) later -- ordered by the data
            # dependency chain within the group, different banks across groups.
            for k in range(NT):
                # output branch (off critical path): yT = u^T x mOut
                yps = pypool.tile([B_LOC, P], f32, tag="yps")
                nc.tensor.matmul(yps[:], u[:], mOut_t[:], start=True, stop=True)
                yc = opool.tile([B_LOC, P], f32, tag="yc")
                nc.scalar.activation(yc[:], yps[:], Copy)
                nc.sync.dma_start(out_d[k + 1], yc[:])

                if k == NT - 1:
                    break

                unew = upool.tile([P, B_LOC], f32, tag="u")
                for g, (c0, c1) in enumerate(GROUPS):
                    z = pzpools[g].tile([P, 64], f32, tag=f"z_{g}")
                    nc.tensor.matmul(z[:], wE_t[:], u[:, c0:c1], start=True, stop=True)
                    nl_group(g, c0, c1, z[:], unew, from_sbuf=False)
                u = unew

    nc.compile()
    return nc


def _get_compiled():
    if "nc" not in _CACHE:
        _CACHE["nc"] = _build_nc()
    return _CACHE["nc"]


def _run(host, trace=False, tmpdir=None):
    from concourse.bass_utils import run_bass_kernel_spmd

    nc = _get_compiled()
    in_maps = []
    for i in range(N_CORES):
        sl = slice(i * B_LOC, (i + 1) * B_LOC)
        in_maps.append({
            "wE": host["wE"],
            "mOut": host["mOut"],
            "z0": np.ascontiguousarray(host["z0"][:, sl]),
            "y0T": np.ascontiguousarray(host["y0T"][sl, :]),
            "ccscale": host["ccscale"],
            "sinscale": host["sinscale"],
        })
    res = run_bass_kernel_spmd(nc, in_maps, list(range(N_CORES)), trace=trace,
                               tmpdir=tmpdir)
    full = np.empty((EVAL_PTS, BATCH, MODES, 2), dtype=np.float32)
    for i in range(N_CORES):
        sl = slice(i * B_LOC, (i + 1) * B_LOC)
        full[:, sl, :, :] = res.results[i]["out"].reshape(EVAL_PTS, B_LOC, MODES, 2)
    return full, res


def kernel(A0, params, omega, kappa, nonlinearity):
    A0 = np.asarray(A0, dtype=np.float32)
    params = np.asarray(params, dtype=np.float32)
    omega = np.asarray(omega, dtype=np.float32)
    kappa = np.asarray(kappa, dtype=np.float32)
    nonlinearity = np.asarray(nonlinearity, dtype=np.float32)

    host = _host_precompute(A0, params, omega, kappa, nonlinearity)
    full, _ = _run(host, trace=False)
    return full


# revision 10
# speedup vs baseline: 1.2649x; 1.0746x over previous
"""Trainium2 kernel for the nn_Circuit coupled-mode ODE problem.

Math: dA/dt = i*diag(omega + gamma*|A|^2) A + T2 A, integrated from t=0 to 2,
sampled at 200 points, with A (1024 batch, 64 modes) complex, A0 padding to
ones for modes 48..63.  T2 (and hence L = T2 + i*diag(omega)) is constant and
nearly skew-Hermitian, with one stiff oscillatory eigenvalue (~288i).

Device algorithm: Strang splitting with the linear part applied EXACTLY via a
host-precomputed matrix exponential (one 128x128 real matmul per step) and the
nonlinear part applied exactly as a per-element phase rotation
A <- A*exp(i*gamma*h*|A|^2) (sin/cos on the scalar engine).  One step per
output interval (h = 2/199).  Using the half-shifted chain state
z_k = E(h/2) y_k:
    u_k    = NL_h(z_k)
    y_{k+1} = E(h/2) u_k          (output branch, transposed+interleaved matmul)
    z_{k+1} = E(h)   u_k          (chain matmul)

State layout on device: (128 partitions, 128 batch) f32 with partition
p = 2j+c interleaving re/im of mode j, so that |A|^2 needs only a pair-swap
stream_shuffle, and the output matmul directly produces the DRAM (batch, j, c)
layout.

Sharding: pure data parallel, batch 1024 = 8 cores x 128.
"""

import os
import numpy as np

MODES = 64
INPUT_MODES = 48
BATCH = 1024
EVAL_PTS = 200
EPS = 1e-8
N_CORES = 8
B_LOC = BATCH // N_CORES  # 128
NT = EVAL_PTS - 1  # 199 intervals
DT = 2.0 / NT

_CACHE = {}


# ---------------------------------------------------------------------------
# host-side math
# ---------------------------------------------------------------------------

def _t2_like_reference(params, omega, kappa):
    """Reproduce the reference's float32 jax computation of T2 exactly."""
    import jax

    try:
        cpu = jax.devices("cpu")[0]
    except Exception:
        cpu = None

    import contextlib

    ctx = jax.default_device(cpu) if cpu is not None else contextlib.nullcontext()
    with ctx:
        import jax.numpy as jnp

        n = MODES
        p = jnp.asarray(params, dtype=jnp.float32)
        n_off = n * (n - 1) // 2
        iu = jnp.triu_indices(n, 1)
        off = p[:n_off] + 1j * p[n_off:2 * n_off]
        H = jnp.zeros((n, n), dtype=jnp.complex64).at[iu].set(off.astype(jnp.complex64))
        H = H + H.conj().T
        d = p[2 * n_off:]
        diag = jnp.concatenate([d, -jnp.sum(d, keepdims=True)])
        H = H + jnp.diag(diag.astype(jnp.complex64))
        U = jax.scipy.linalg.expm(1j * H)
        I = jnp.eye(n, dtype=jnp.complex64)
        M = U.T @ U
        mix = M @ jnp.linalg.inv(I - M + EPS * I)
        T2 = -jnp.asarray(kappa, dtype=jnp.float32) * (
            0.5 * jnp.eye(n, dtype=jnp.float32) + mix
        )
        T2_re = np.asarray(jnp.real(T2), dtype=np.float32)
        T2_im = np.asarray(jnp.imag(T2), dtype=np.float32)
    return T2_re, T2_im


def _expm(M):
    """Matrix exponential of a (diagonalizable) complex matrix via eig."""
    w, V = np.linalg.eig(M)
    return (V * np.exp(w)) @ np.linalg.inv(V)


def _big_il(C):
    """Complex (64,64) -> real (128,128) operator in the interleaved re/im basis."""
    A = np.zeros((2 * MODES, 2 * MODES), dtype=np.float64)
    Cr, Ci = C.real, C.imag
    A[0::2, 0::2] = Cr
    A[0::2, 1::2] = -Ci
    A[1::2, 0::2] = Ci
    A[1::2, 1::2] = Cr
    return A


def _host_precompute(A0, params, omega, kappa, nonlinearity):
    T2_re, T2_im = _t2_like_reference(params, omega, kappa)
    L = T2_re.astype(np.float64) + 1j * T2_im.astype(np.float64)
    L = L + 1j * np.diag(omega.astype(np.float64))

    E1 = _expm(L * DT)          # full-step propagator
    E2 = _expm(L * (DT / 2))    # half-step propagator

    A1 = _big_il(E1)
    A2 = _big_il(E2)
    perm = np.arange(128) ^ 1   # pair swap

    wE = np.ascontiguousarray(A1.T, dtype=np.float32)
    wEsw = np.ascontiguousarray(A1[perm, :].T, dtype=np.float32)
    mOut = np.ascontiguousarray(A2.T, dtype=np.float32)

    # initial state, interleaved: (128, BATCH)
    y0 = np.zeros((2 * MODES, BATCH), dtype=np.float64)
    y0[0:2 * INPUT_MODES:2, :] = A0[:, :, 0].astype(np.float64).T
    y0[1:2 * INPUT_MODES:2, :] = A0[:, :, 1].astype(np.float64).T
    y0[2 * INPUT_MODES::2, :] = 1.0
    z0 = (A2 @ y0).astype(np.float32)
    y0T = np.ascontiguousarray(y0.T, dtype=np.float32)  # (BATCH, 128)

    gh = (nonlinearity.astype(np.float64) * DT)  # per-mode gamma*h
    # SSp = sin(theta) with sign (+ on even partitions, - on odd): the rotation
    # cross-term is built as q = pairswap(z * SSp).
    sgn = np.tile([1.0, -1.0], MODES)
    sinscale = (np.repeat(gh, 2) * sgn).astype(np.float32).reshape(128, 1)
    # cc = 1 - theta^2/2 = msq * (-(gamma*h)^2/2) + 1, per-partition coefficient
    ccscale = (-np.repeat(gh, 2) ** 2 / 2).astype(np.float32).reshape(128, 1)

    return dict(wE=wE, wEsw=wEsw, mOut=mOut, z0=z0, y0T=y0T,
                ccscale=ccscale, sinscale=sinscale)


# ---------------------------------------------------------------------------
# device kernel
# ---------------------------------------------------------------------------

def _build_nc():
    import concourse.bass as bass
    import concourse.bacc as bacc
    import concourse.tile as tile
    import concourse.mybir as mybir

    f32 = mybir.dt.float32
    Sin = mybir.ActivationFunctionType.Sin
    Square = mybir.ActivationFunctionType.Square
    Copy = mybir.ActivationFunctionType.Copy
    add = mybir.AluOpType.add
    mult = mybir.AluOpType.mult
    P = 128
    pairswap = [i ^ 1 for i in range(32)]

    nc = bacc.Bacc("TRN2", target_bir_lowering=False, debug=False,
                   num_devices=N_CORES)

    wE_d = nc.dram_tensor("wE", [P, P], f32, kind="ExternalInput").ap()
    mOut_d = nc.dram_tensor("mOut", [P, P], f32, kind="ExternalInput").ap()
    z0_d = nc.dram_tensor("z0", [P, B_LOC], f32, kind="ExternalInput").ap()
    y0T_d = nc.dram_tensor("y0T", [B_LOC, P], f32, kind="ExternalInput").ap()
    ccscale_d = nc.dram_tensor("ccscale", [P, 1], f32, kind="ExternalInput").ap()
    sinscale_d = nc.dram_tensor("sinscale", [P, 1], f32, kind="ExternalInput").ap()
    out_d = nc.dram_tensor("out", [EVAL_PTS, B_LOC, P], f32, kind="ExternalOutput").ap()

    GROUPS = [(0, 64), (64, 128)]

    with tile.TileContext(nc) as tc:
        with (
            tc.tile_pool(name="const", bufs=1) as cpool,
            tc.tile_pool(name="u", bufs=4) as upool,
            tc.tile_pool(name="nl", bufs=4) as npool,
            tc.tile_pool(name="oy", bufs=4) as opool,
            tc.tile_pool(name="pz0", bufs=2, space="PSUM") as pz0pool,
            tc.tile_pool(name="pz1", bufs=2, space="PSUM") as pz1pool,
            tc.tile_pool(name="py", bufs=2, space="PSUM") as pypool,
        ):
            wE_t = cpool.tile([P, P], f32, tag="wE")
            mOut_t = cpool.tile([P, P], f32, tag="mOut")
            ccsc_t = cpool.tile([P, 1], f32, tag="ccsc")
            sinsc_t = cpool.tile([P, 1], f32, tag="sinsc")
            nc.sync.dma_start(wE_t[:], wE_d[:])
            nc.sync.dma_start(mOut_t[:], mOut_d[:])
            nc.sync.dma_start(ccsc_t[:], ccscale_d[:])
            nc.sync.dma_start(sinsc_t[:], sinscale_d[:])

            # t=0 output: pass-through of the initial state
            y0_t = opool.tile([B_LOC, P], f32, tag="yc")
            nc.sync.dma_start(y0_t[:], y0T_d[:])
            nc.sync.dma_start(out_d[0], y0_t[:])

            pzpools = [pz0pool, pz1pool]

            def nl_group(g, c0, c1, zsrc, unew, from_sbuf):
                """u'[:, c0:c1] = exp(i*theta)*z from zsrc (slice or tile)."""
                FD = c1 - c0
                s2 = npool.tile([P, FD], f32, tag=f"s2_{g}")
                s2sw = npool.tile([P, FD], f32, tag=f"s2sw_{g}")
                m2 = npool.tile([P, FD], f32, tag=f"m2_{g}")
                msq = npool.tile([P, FD], f32, tag=f"msq_{g}")
                cc = npool.tile([P, FD], f32, tag=f"cc_{g}")
                ssp = npool.tile([P, FD], f32, tag=f"ssp_{g}")
                pp = npool.tile([P, FD], f32, tag=f"pp_{g}")
                qt = npool.tile([P, FD], f32, tag=f"qt_{g}")
                qq = npool.tile([P, FD], f32, tag=f"qq_{g}")
                if from_sbuf:
                    nc.vector.tensor_tensor(s2[:], zsrc, zsrc, mult)
                else:
                    nc.scalar.activation(s2[:], zsrc, Square)
                nc.vector.stream_shuffle(s2sw[:], s2[:], pairswap)
                nc.vector.tensor_tensor(m2[:], s2[:], s2sw[:], add)
                nc.scalar.activation(ssp[:], m2[:], Sin, scale=sinsc_t[:])
                # cos branch off the critical path (runs during the Sin LUT)
                nc.gpsimd.tensor_tensor(msq[:], m2[:], m2[:], mult)
                nc.gpsimd.tensor_scalar(cc[:], msq[:], ccsc_t[:], 1.0, mult, add)
                nc.vector.tensor_tensor(pp[:], zsrc, cc[:], mult)
                nc.vector.tensor_tensor(qt[:], zsrc, ssp[:], mult)
                nc.vector.stream_shuffle(qq[:], qt[:], pairswap)
                nc.vector.tensor_tensor(unew[:, c0:c1], pp[:], qq[:], add)

            # ---- initial nonlinear step: u_0 = NL(z0), z0 from DRAM ----
            z0_t = upool.tile([P, B_LOC], f32, tag="z0src")
            nc.sync.dma_start(z0_t[:], z0_d[:])
            u = upool.tile([P, B_LOC], f32, tag="u")
            for g, (c0, c1) in enumerate(GROUPS):
                nl_group(g, c0, c1, z0_t[:, c0:c1], u, from_sbuf=True)

            # ---- main loop: two independent per-group chains ----
            # PSUM single-reader discipline: each z_g PSUM tile is read by ACT
            # (Square) first and V (pp/qt muls) later -- ordered by the data
            # dependency chain within the group, different banks across groups.
            for k in range(NT):
                # chain first: its matmuls must lead the PE FIFO each interval
                if k < NT - 1:
                    unew = upool.tile([P, B_LOC], f32, tag="u")
                    zs = []
                    for g, (c0, c1) in enumerate(GROUPS):
                        z = pzpools[g].tile([P, 64], f32, tag=f"z_{g}")
                        nc.tensor.matmul(z[:], wE_t[:], u[:, c0:c1],
                                         start=True, stop=True)
                        zs.append(z)
                    for g, (c0, c1) in enumerate(GROUPS):
                        nl_group(g, c0, c1, zs[g][:], unew, from_sbuf=False)

                # output branch (off critical path): yT = u^T x mOut
                yps = pypool.tile([B_LOC, P], f32, tag="yps")
                nc.tensor.matmul(yps[:], u[:], mOut_t[:], start=True, stop=True)
                yc = opool.tile([B_LOC, P], f32, tag="yc")
                nc.scalar.activation(yc[:], yps[:], Copy)
                nc.sync.dma_start(out_d[k + 1], yc[:])

                if k == NT - 1:
                    break
                u = unew

    nc.compile()
    return nc


def _get_compiled():
    if "nc" not in _CACHE:
        _CACHE["nc"] = _build_nc()
    return _CACHE["nc"]


def _run(host, trace=False, tmpdir=None):
    from concourse.bass_utils import run_bass_kernel_spmd

    nc = _get_compiled()
    in_maps = []
    for i in range(N_CORES):
        sl = slice(i * B_LOC, (i + 1) * B_LOC)
        in_maps.append({
            "wE": host["wE"],
            "mOut": host["mOut"],
            "z0": np.ascontiguousarray(host["z0"][:, sl]),
            "y0T": np.ascontiguousarray(host["y0T"][sl, :]),
            "ccscale": host["ccscale"],
            "sinscale": host["sinscale"],
        })
    res = run_bass_kernel_spmd(nc, in_maps, list(range(N_CORES)), trace=trace,
                               tmpdir=tmpdir)
    full = np.empty((EVAL_PTS, BATCH, MODES, 2), dtype=np.float32)
    for i in range(N_CORES):
        sl = slice(i * B_LOC, (i + 1) * B_LOC)
        full[:, sl, :, :] = res.results[i]["out"].reshape(EVAL_PTS, B_LOC, MODES, 2)
    return full, res


def kernel(A0, params, omega, kappa, nonlinearity):
    A0 = np.asarray(A0, dtype=np.float32)
    params = np.asarray(params, dtype=np.float32)
    omega = np.asarray(omega, dtype=np.float32)
    kappa = np.asarray(kappa, dtype=np.float32)
    nonlinearity = np.asarray(nonlinearity, dtype=np.float32)

    host = _host_precompute(A0, params, omega, kappa, nonlinearity)
    full, _ = _run(host, trace=False)
    return full
